# revision 10
# baseline (speedup 1.0000x reference)
"""Bbox regression loss (smooth-L1 over gathered bbox deltas) on 8 TRN2 cores.

The loss gathers 4 scalars per (batch, gt-box) from each FPN level's dense
prediction tensor, applies smooth-L1 against the gt deltas, and reduces to
two scalars (weighted loss sum, valid-box count).  Only 3 x 2 x 128 x 4 =
3072 elements of the ~92MB of predictions are ever read, so the kernel is
built around one on-device dma_gather rather than streaming.

Sharding: core c handles (b = c//4, k = c%4) where k indexes the 4 bbox
coordinate channels (channel group k*A:(k+1)*A of the 4*A=12 channel dim).
Each core receives exactly 1/8 of every prediction tensor (concatenated
into one row table), computes its partial (loss, weight) fully on device,
and the host sums the 8 partials.

Device pipeline per core (critical path = 3 chained DMAs, everything else
is hidden):
  1. aux load via a PREPARE_ONLY SWDGE gather with static iota indices:
     the descriptor-gen runs before the program's start barrier and the
     trigger fires immediately, skipping the HWDGE + DGE-delay fixed costs
     of a regular dma_start.  The aux row per gt-entry carries gt deltas,
     validity, in-row element offsets (rem) and the packed int16 gather row
     indices -- all precomputed on host from the (small) coord tensors.
     Masked entries (pad gt or inactive sample) are pointed at a zero pad
     row appended to the table with gt=0, so they contribute exactly 0 loss
     with no on-device masking.
  2. main dma_gather (PREPARE_ONLY + trigger) fetches 384 512B rows from
     the concatenated prediction table -> g[m, level, 128] f32.
  3. fused scalar_tensor_tensor one-hot select (iota==rem)*g with
     per-partition accumulate -> pred[m,l]; smooth-L1 via the identity
     2*sl(d) = min(|d|,1) * max(2|d|-1, |d|) (the 0.5 folded into the
     host-side loss weight); result written next to the validity columns.
  4. output via a PREPARE_ONLY dma_scatter_add whose 128 indices all hit
     row 0 of the (pre-zeroed) output: the DMA engine itself performs the
     partition reduction, replacing the PE matmul + PSUM copy + HWDGE
     output DMA with a single trigger fired right after the last vector op.
"""

import os

import numpy as np

try:  # persistent XLA/NEFF compile cache across processes
    import jax

    os.makedirs("/tmp/jax_pcache", exist_ok=True)
    jax.config.update("jax_compilation_cache_dir", "/tmp/jax_pcache")
    jax.config.update("jax_persistent_cache_min_compile_time_secs", 0.0)
    jax.config.update("jax_persistent_cache_min_entry_size_bytes", 0)
except Exception:
    pass

import concourse.bacc as bacc
import concourse.bass as bass
import concourse.tile as tile
from concourse import mybir
from concourse.bass_utils import run_bass_kernel_spmd

A = 3                       # anchors per level
M = 128                     # gt entries per sample
GRIDS = (96, 48, 24)        # level l grid; level l uses coord/diff index 2-l
LOSS_W = (1.0, 1.0, 1.0, 0.1)
ROW = 128                   # f32 elements per gather row (512B)
NLVL = 3
NIDX = NLVL * M             # 384 gathered rows per core
V = tuple(A * g * g * g // ROW for g in GRIDS)      # (20736, 2592, 324)
VBASE = (0, V[0], V[0] + V[1])
VTOT = sum(V)               # 23652 rows; +1 zero pad row < int16 max
N_CORES = 8

AUXC = 64                   # aux row: 256B gather granularity
# aux f32 columns: 0:3 gt | 3:6 validf | 6:9 remf | 10:22 idx16 (bitcast)
IDXC = 10

F32 = mybir.dt.float32
I16 = mybir.dt.int16
Alu = mybir.AluOpType


def _build_bass() -> bass.Bass:
    nc = bacc.Bacc(
        "TRN2",
        target_bir_lowering=False,
        debug=False,
        num_devices=N_CORES,
        num_swdge_queues=3,
    )
    tab = nc.dram_tensor("tab", [VTOT + 1, ROW], F32, kind="ExternalInput")
    auxi = nc.dram_tensor("auxi", [M, 16], F32, kind="ExternalInput")
    auxd = nc.dram_tensor("aux", [M, AUXC], F32, kind="ExternalInput")
    out = nc.dram_tensor("partial", [1, AUXC], F32, kind="ExternalOutput")

    with tile.TileContext(nc) as tc:
        with tc.tile_pool(name="sb", bufs=1) as sb:
            aux = sb.tile([M, AUXC], F32)
            auxit = sb.tile([M, 16], F32)
            g = sb.tile([M, NLVL, ROW], F32)
            io = sb.tile([M, ROW], F32)
            ones = sb.tile([M, ROW], F32)
            zi = sb.tile([M, NIDX // 16 // 3], I16)   # [128, 8] zeros
            pred = sb.tile([M, NLVL], F32)
            scr0 = sb.tile([M, ROW], F32)
            scr1 = sb.tile([M, ROW], F32)
            d = sb.tile([M, NLVL], F32)
            pmin = sb.tile([M, NLVL], F32)
            t1 = sb.tile([M, NLVL], F32)
            q = sb.tile([M, NLVL], F32)

            # --- aux loads via HWDGE from SP: for the head-of-program DMA
            # (no waits) SEQ+HWDGE gen overlap the start barrier, beating a
            # SWDGE prep+trigger.  The 48B idx payload goes first/alone so
            # the gather prep can start ~125ns earlier. ---
            nc.sync.dma_start(out=auxit[:], in_=auxi[:])
            nc.sync.dma_start(out=aux[:], in_=auxd[:])

            # constants on the (otherwise idle) DVE: io = iota via prefix
            # scan of ones, zi = the scatter's all-zero index block
            nc.vector.memset(ones[:], 1.0)
            nc.vector.tensor_tensor_scan(
                io[:], ones[:], ones[:], -1.0, Alu.add, Alu.bypass
            )
            nc.vector.memset(zi[:], 0)

            # --- main gather: 384 rows of 512B; prep waits only on auxi ---
            idx16 = auxit[:, 0 : NIDX // 16 // 2].bitcast(I16)
            nc.gpsimd.dma_gather(
                g[:], tab[:], idx16, NIDX, NIDX, ROW,
                prepare_only=True, queue_num=0,
                sem=nc.alloc_semaphore("g_dma"),
            )
            nc.gpsimd.trigger_dma(count=None, queue_num=0)

            # --- output scatter-add: all 128 idx hit row 0 (the DMA is the
            # partition reduction); prep in the gather-transfer window,
            # trigger after the last vector op ---
            aux3 = aux[:].rearrange("p (a f) -> p a f", a=1)
            nc.gpsimd.dma_scatter_add(
                out[:], aux3, zi[:], M, M, AUXC,
                prepare_only=True, queue_num=1,
                sem=nc.alloc_semaphore("out_dma"),
            )

            # pred[m,l] = g[m,l,rem[m,l]] -- fused (iota==rem)*g + row-sum.
            # All on DVE: per-partition-scalar ops (TensorScalarPtr) only
            # exist there (walrus rejects them on Pool).
            gts = aux[:, 0:3]
            remf = aux[:, 6:9]
            for lvl in (0, 1, 2):
                nc.vector.scalar_tensor_tensor(
                    out=scr0[:],
                    in0=io[:],
                    scalar=remf[:, lvl : lvl + 1],
                    in1=g[:, lvl, :],
                    op0=Alu.is_equal,
                    op1=Alu.mult,
                    accum_out=pred[:, lvl : lvl + 1],
                )

            # smooth l1 (x2) in 5 ops via
            #   2*sl(d) = (relu(|d|-1) + (|d|-1) + 1) * min(|d|,1)
            # (|d|<1: |d|*|d|; |d|>=1: (2|d|-1)*1; the 0.5 in host wk)
            nc.vector.tensor_tensor(d[:], pred[:], gts, Alu.subtract)
            nc.vector.tensor_scalar(t1[:], d[:], 0.0, 1.0, Alu.abs_max, Alu.subtract)
            nc.vector.scalar_tensor_tensor(
                out=q[:], in0=t1[:], scalar=0.0, in1=t1[:],
                op0=Alu.max, op1=Alu.add,
            )
            nc.vector.tensor_scalar(pmin[:], d[:], 0.0, 1.0, Alu.abs_max, Alu.min)
            # sl2 lands in aux[:,0:3], next to validf in 3:6; junk in the
            # remaining columns is summed into out[0, 6:] which is unread.
            nc.vector.scalar_tensor_tensor(
                out=aux[:, 0:3], in0=q[:], scalar=1.0, in1=pmin[:],
                op0=Alu.add, op1=Alu.mult,
            )
            nc.gpsimd.trigger_dma(count=None, queue_num=1)

    # Tile assigns each DMA a DMASW lane tick and points every consumer wait
    # at the lane semaphore, but for PREPARE_ONLY preps it leaves the user
    # `sem=` as on_update[0] (the slot both hardware SDMA and the sim bump on
    # DMA completion).  Repoint on_update[0] at the lane semaphore so the
    # completion actually satisfies the consumers.
    from concourse.tile_scheduler import PROC_NAMES

    fn = nc.m.functions[0]
    lane_sem: dict[str, tuple[int, str]] = {}
    for bb in fn.blocks:
        for ins in bb.instructions:
            si = ins.sync_info
            if si is None:
                continue
            for w in si.on_wait:
                if w.ant_name and w.ant_name.startswith("DMASW"):
                    lane_sem[w.ant_name.split("_")[0]] = (w.id, w.ant_name)
    for bb in fn.blocks:
        for ins in bb.instructions:
            if getattr(ins, "gen_mode", 0) != 1:
                continue
            lane = PROC_NAMES[ins.bass_scheduled_proc]
            assert lane.startswith("DMASW"), lane
            sem_id, sem_name = lane_sem[lane]
            u0 = ins.sync_info.on_update[0]
            u0.id = sem_id
            u0.ant_name = sem_name
    nc.finalize()
    return nc


_NC = None


def _get_nc():
    global _NC
    if _NC is None:
        _NC = _build_bass()
    return _NC


def kernel(**inputs: np.ndarray):
    out_l = [np.asarray(inputs[n]) for n in ("out1", "out3", "out5")]
    # level l uses coord/diff (2-l)  (the reference pairs them reversed)
    coords = [np.asarray(inputs[f"coord{2 - l}"]) for l in range(3)]
    diffs = [np.asarray(inputs[f"diff{2 - l}"]) for l in range(3)]

    in_maps = []
    for c in range(N_CORES):
        b, k = c // 4, c % 4
        im = {}
        im["tab"] = np.concatenate(
            [
                np.ascontiguousarray(out_l[l][b, A * k : A * (k + 1)]).reshape(
                    V[l], ROW
                )
                for l in range(3)
            ]
            + [np.zeros((1, ROW), np.float32)],
            axis=0,
        )
        aux = np.zeros((M, AUXC), np.float32)
        rows = np.zeros((M, NLVL), np.int64)
        for l, g in enumerate(GRIDS):
            cc = coords[l][b].astype(np.int64)  # [128, 4]
            valid = cc[:, 0] > -1
            active = bool(cc[0, 0] > -1)
            mask = valid & active
            a = np.maximum(cc[:, 0], 0)
            flat = ((a * g + cc[:, 1]) * g + cc[:, 2]) * g + cc[:, 3]
            row = VBASE[l] + (flat >> 7)
            rem = flat & (ROW - 1)
            row[~mask] = VTOT          # zero pad row
            rem[~mask] = 0
            rows[:, l] = row
            aux[:, 6 + l] = rem.astype(np.float32)
            aux[:, 3 + l] = mask.astype(np.float32)
            aux[mask, l] = diffs[l][b, mask, k]
        # wrapped idx layout: item i = l*128 + m -> idxw[i%16, i//16],
        # replicated across the 8 gpsimd cores (partition p reads p%16 row)
        idxw = np.zeros((16, NIDX // 16), np.int16)
        for l in range(NLVL):
            for mm in range(M):
                i = l * M + mm
                idxw[i % 16, i // 16] = rows[mm, l]
        auxi = np.zeros((M, 16), np.float32)
        auxi[:, 0 : NIDX // 16 // 2] = np.tile(idxw, (8, 1)).view(np.float32)
        im["auxi"] = auxi
        im["aux"] = aux
        in_maps.append(im)

    res = run_bass_kernel_spmd(_get_nc(), in_maps, core_ids=list(range(N_CORES)))
    # host epilogue of the reduction: per-core constant loss-weight scaling
    # (0.5*LOSS_W[k], weight counted once via the k==0 cores) + all-reduce
    loss = np.float32(0.0)
    weight = np.float32(0.0)
    for c in range(N_CORES):
        k = c % 4
        p6 = res.results[c]["partial"][0]
        loss += np.float32(p6[0:3].sum() * np.float32(0.5 * LOSS_W[k]))
        if k == 0:
            weight += np.float32(p6[3:6].sum())
    return (np.array([loss], np.float32), np.array([weight], np.float32))
